# revision 1
# baseline (speedup 1.0000x reference)
"""nn_ACBlock (conv + 16x16 window attention block) on 8 Trainium2 NeuronCores.

Self-contained kernel: kernel(**inputs) takes the FULL problem inputs
(x [4,128,256,256] plus weights/biases as in setup_inputs), shards them
data-parallel over 8 cores (batch x vertical half; bottom halves are
vertically flipped together with the conv weights so a single SPMD graph
serves every core), runs a bass/Tile kernel via run_bass_kernel_spmd,
and reassembles the full [4,128,256,256] float32 output.
"""
import sys
sys.path.insert(0, '/opt/trn_rl_repo')
import numpy as np
import ml_dtypes

import concourse.bass as bass
import concourse.mybir as mybir
import concourse.tile as tile
from concourse.masks import make_identity
from concourse.bass_utils import run_bass_kernel_spmd

bf16 = ml_dtypes.bfloat16

# ------------------------- walrus compatibility -------------------------
FixedTileContext = tile.TileContext


FixedTileContext = tile.TileContext

def split_excess_waits(nc):
    """Walrus in this env accepts only one sync wait per instruction.
    Move excess waits onto injected per-engine NoOps placed just before
    the over-subscribed instruction (same engine => sequenced waits are
    semantically identical to combined waits)."""
    k = 0
    for fn in nc.m.functions:
        for blk in fn.blocks:
            il = blk.instructions
            out = []
            changed = False
            for inst in il:
                si = getattr(inst, 'sync_info', None)
                waits = list(si.on_wait) if (si is not None and si.on_wait) else []
                if len(waits) > 1:
                    changed = True
                    for w in waits[:-1]:
                        nop = mybir.InstNoOp(name=f"I-wsplit-{k}", ins=[], outs=[])
                        k += 1
                        nop.engine = inst.engine
                        nop.bass_nofuse = True
                        nop.sync_info = mybir.SyncInfo(on_wait=[w], on_update=[])
                        out.append(nop)
                    inst.sync_info = mybir.SyncInfo(on_wait=[waits[-1]],
                                                    on_update=list(si.on_update))
                out.append(inst)
            if changed:
                blk.instructions = out


# ------------------------- device graph builder -------------------------

F32 = mybir.dt.float32
BF = mybir.dt.bfloat16
AF = mybir.ActivationFunctionType
ALU = mybir.AluOpType

C = 128
W = 256
BS = 16
NBR = 9          # block-rows of attention (rows [0,144))
R_X = 148
R_C1 = 132
R_CNN = 131
R_F = 130
R_G1 = 129
R_OUT = 128

# dw-conv row split per 16-row block-row across engines
DW_PE_ROWS = 6    # diagonal matmuls on TensorE
DW_ACT_ROWS = 0   # scaled-copy on ScalarE + 4x add on VectorE
# remaining rows: scalar_tensor_tensor FMA chain on VectorE

TAPS3 = [(1, 1)] + [(dy, dx) for dy in range(3) for dx in range(3) if (dy, dx) != (1, 1)]


def build(dw_pe_rows=DW_PE_ROWS, dw_act_rows=DW_ACT_ROWS):
    nc = bass.Bass()
    ext = {}
    ext["xbf"] = nc.declare_dram_parameter("xbf", [C, R_X, W], BF, isOutput=False)
    ext["xres"] = nc.declare_dram_parameter("xres", [C, R_OUT, W], F32, isOutput=False)
    for name in ("r1w1", "r1w2", "fw1", "fw2", "r2w1", "r2w2"):
        ext[f"w_{name}"] = nc.declare_dram_parameter(f"w_{name}", [9, C, C], BF,
                                                     isOutput=False)
    for name in ("aw2", "aw3", "aw4"):
        ext[f"w_{name}"] = nc.declare_dram_parameter(f"w_{name}", [C, C], BF,
                                                     isOutput=False)
    ext["awdw"] = nc.declare_dram_parameter("awdw", [C, 49], F32, isOutput=False)
    # bias cols: 0 r1b1, 1 r1b2, 2 ab1, 3 ab2, 4 ab3s, 5 ab4, 6 fb, 7 r2b1, 8 r2b2
    ext["biases"] = nc.declare_dram_parameter("biases", [C, 9], F32, isOutput=False)
    ext["out"] = nc.declare_dram_parameter("out", [C, R_OUT, W], F32, isOutput=True)

    with FixedTileContext(nc) as tc:
        _build_body(nc, tc, ext, dw_pe_rows, dw_act_rows)
    split_excess_waits(nc)
    return nc


def _build_body(nc, tc, ext, dw_pe_rows, dw_act_rows):
    xbf, xres, out_ext = ext["xbf"], ext["xres"], ext["out"]
    from contextlib import ExitStack
    ctx = ExitStack()
    with ctx:
        const = ctx.enter_context(tc.tile_pool(name="const", bufs=1))
        dram = ctx.enter_context(tc.tile_pool(name="dram", bufs=1, space="DRAM"))
        apool = ctx.enter_context(tc.tile_pool(name="apool", bufs=2))
        spool = ctx.enter_context(tc.tile_pool(name="spool", bufs=2))
        cpool = ctx.enter_context(tc.tile_pool(name="cpool", bufs=2))
        cpsum = ctx.enter_context(tc.tile_pool(name="cpsum", bufs=3, space="PSUM"))
        lpsum = ctx.enter_context(tc.tile_pool(name="lpsum", bufs=2, space="PSUM"))
        tpsum = ctx.enter_context(tc.tile_pool(name="tpsum", bufs=1, space="PSUM"))
        xpsum = ctx.enter_context(tc.tile_pool(name="xpsum", bufs=2, space="PSUM"))

        # ---- persistent weights / constants
        # small tensors + diag build first: the depthwise diagonal matmuls
        # are TensorE's first work and must not queue behind the ~2MB of
        # strided weight-stack DMAs
        awdw_sb = const.tile([C, 49], F32, tag="awdw")
        nc.sync.dma_start(awdw_sb[:], ext["awdw"][:])
        bias_sb = const.tile([C, 9], F32, tag="bias")
        nc.sync.dma_start(bias_sb[:], ext["biases"][:])

        def bias_ap(i):
            return bias_sb[:, i:i + 1]

        ident = const.tile([C, C], BF, tag="ident")
        make_identity(nc, ident[:])
        diag = const.tile([C, 49, C], BF, tag="diag")
        for t in range(49):
            nc.vector.tensor_scalar_mul(diag[:, t, :], ident[:], awdw_sb[:, t:t + 1])
        wt = {}
        for name in ("r1w1", "r1w2", "fw1", "fw2", "r2w1", "r2w2"):
            t = const.tile([C, 9, C], BF, tag=f"wt_{name}")
            nc.sync.dma_start(t[:], ext[f"w_{name}"].rearrange("t i o -> i t o"))
            wt[name] = t
        aw = {}
        for name in ("aw2", "aw3", "aw4"):
            t = const.tile([C, C], BF, tag=f"wt_{name}")
            nc.sync.dma_start(t[:], ext[f"w_{name}"][:])
            aw[name] = t

        # dw-conv input slabs: 22 rows x 264 cols; cols [4,260) hold data,
        # outer cols stay zero (width zero-pad). Two slots. xodd is a
        # one-element-shifted copy so taps with odd column offsets keep the
        # 4B-aligned APs the DVE 2x/4x modes need.
        xchunk = const.tile([C, 2, 23, 264], BF, tag="xchunk")
        nc.vector.memset(xchunk[:], 0.0)
        xodd = const.tile([C, 23, 264], BF, tag="xodd")
        nc.vector.memset(xodd[:], 0.0)

        # ---- DRAM intermediates (row-major)
        c1_d = dram.tile([C, R_C1, W], BF, tag="c1_d")
        cnn_d = dram.tile([C, R_CNN, W], BF, tag="cnn_d")
        xo_d = dram.tile([C, NBR * BS, W], BF, tag="xo_d")
        f_d = dram.tile([C, R_F, W], BF, tag="f_d")
        g1_d = dram.tile([C, R_G1, W], BF, tag="g1_d")

        # ================= conv machinery =================
        def conv_chunk(kc, srcs, wts, R_out, R_in, evac, stag=""):
            """One 2-row (or trailing 1-row) chunk of a 3x3 conv."""
            r0 = 2 * kc
            ncr = min(2, R_out - r0)
            ins = []
            for si, src in enumerate(srcs):
                it = cpool.tile([C, 4, W], BF, tag=f"cin{si}{stag}",
                                bufs=3 if stag in ("a", "b") else 2)
                ilo = max(0, r0 - 1)
                ihi = min(R_in, r0 + ncr + 1)
                nc.sync.dma_start(it[:, ilo - (r0 - 1):ihi - (r0 - 1), :],
                                  src[:, ilo:ihi, :])
                ins.append(it)
            ps = cpsum.tile([C, 2, W], F32, tag="cps")
            # full-width dx=1 taps as flat [rows*W] runs; dx!=1 taps split
            # per output row (walrus: 1 free dim per matmul AP)
            mms = []
            for gi in range(len(srcs)):
                for (dy, dx) in (TAPS3 if gi == 0 else
                                 [(d, e) for d in range(3) for e in range(3)]):
                    rlo = max(0, 1 - dy - r0)
                    rhi = min(ncr, R_in - (r0 + dy - 1))
                    if rhi <= rlo:
                        continue
                    wv = wts[gi][:, dy * 3 + dx, :]
                    if dx == 1:
                        mms.append((ps[:, rlo:rhi, :], wv,
                                    ins[gi][:, dy + rlo:dy + rhi, :]))
                    else:
                        clo = max(0, 1 - dx)
                        chi = W - max(0, dx - 1)
                        for r in range(rlo, rhi):
                            mms.append((ps[:, r, clo:chi], wv,
                                        ins[gi][:, dy + r,
                                                clo + dx - 1:chi + dx - 1]))
            for i, (o, wv, rhs) in enumerate(mms):
                nc.tensor.matmul(o, wv, rhs, start=(i == 0),
                                 stop=(i == len(mms) - 1))
            evac(ps, r0, ncr)

        def evac_conv1(ps, r0, ncr):
            o = cpool.tile([C, 2, W], BF, tag="c1o")
            nc.scalar.activation(o[:, :ncr, :], ps[:, :ncr, :], AF.Relu,
                                 bias=bias_ap(0))
            nc.sync.dma_start(c1_d[:, r0:r0 + ncr, :], o[:, :ncr, :])

        def evac_cnn(ps, r0, ncr):
            xt = cpool.tile([C, 2, W], BF, tag="cnnx")
            nc.sync.dma_start(xt[:, :ncr, :], xbf[:, r0:r0 + ncr, :])
            o = cpool.tile([C, 2, W], BF, tag="cnno")
            nc.vector.scalar_tensor_tensor(o[:, :ncr, :], ps[:, :ncr, :],
                                           bias_ap(1), xt[:, :ncr, :],
                                           op0=ALU.add, op1=ALU.add)
            nc.sync.dma_start(cnn_d[:, r0:r0 + ncr, :], o[:, :ncr, :])

        def evac_fused(ps, r0, ncr):
            o = cpool.tile([C, 2, W], BF, tag="fo")
            nc.scalar.activation(o[:, :ncr, :], ps[:, :ncr, :], AF.Identity,
                                 bias=bias_ap(6))
            nc.sync.dma_start(f_d[:, r0:r0 + ncr, :], o[:, :ncr, :])

        def evac_g1(ps, r0, ncr):
            o = cpool.tile([C, 2, W], BF, tag="g1o")
            nc.scalar.activation(o[:, :ncr, :], ps[:, :ncr, :], AF.Relu,
                                 bias=bias_ap(7))
            nc.sync.dma_start(g1_d[:, r0:r0 + ncr, :], o[:, :ncr, :])

        def evac_out(ps, r0, ncr):
            ft = cpool.tile([C, 2, W], BF, tag="fin")
            nc.sync.dma_start(ft[:, :ncr, :], f_d[:, r0:r0 + ncr, :])
            xrt = cpool.tile([C, 2, W], F32, tag="xrin")
            nc.sync.dma_start(xrt[:, :ncr, :], xres[:, r0:r0 + ncr, :])
            t32 = cpool.tile([C, 2, W], F32, tag="t32")
            nc.vector.scalar_tensor_tensor(t32[:, :ncr, :], ps[:, :ncr, :],
                                           bias_ap(8), ft[:, :ncr, :],
                                           op0=ALU.add, op1=ALU.add)
            nc.vector.tensor_tensor(t32[:, :ncr, :], t32[:, :ncr, :],
                                    xrt[:, :ncr, :], ALU.add)
            nc.sync.dma_start(out_ext[:, r0:r0 + ncr, :], t32[:, :ncr, :])

        STAGES = {
            "c1": ([xbf], ["r1w1"], R_C1, min(R_X, R_C1 + 1), evac_conv1, "a"),
            "c2": ([c1_d], ["r1w2"], R_CNN, R_C1, evac_cnn, "b"),
            "fu": ([cnn_d, xo_d], ["fw1", "fw2"], R_F, R_CNN, evac_fused, "c"),
            "r2a": ([f_d], ["r2w1"], R_G1, R_F, evac_g1, "d"),
            "r2b": ([g1_d], ["r2w2"], R_OUT, R_G1, evac_out, "e"),
        }
        NCHUNK = {s: (STAGES[s][2] + 1) // 2 for s in STAGES}
        cur = {s: 0 for s in STAGES}

        def eligible(stage, kc, xo_rows):
            # chunk kc covers out rows [2kc, 2kc+2); input rows [2kc-1, 2kc+3)
            need = 2 * kc + 3
            if stage == "c1":
                return True
            if stage == "c2":
                return cur["c1"] * 2 >= min(need, R_C1)
            if stage == "fu":
                return (cur["c2"] * 2 >= min(need, R_CNN)
                        and xo_rows >= min(need, R_CNN))
            if stage == "r2a":
                return cur["fu"] * 2 >= min(need, R_F)
            if stage == "r2b":
                return cur["r2a"] * 2 >= min(need, R_G1)

        def emit_convs(xo_rows, targets, max_chunks=10 ** 9):
            emitted = 0
            for s in ("c1", "c2", "fu", "r2a", "r2b"):
                srcs, wns, R_out, R_in, evac, stag = STAGES[s]
                wts = [wt[w] for w in wns]
                tgt = min(NCHUNK[s], targets.get(s, NCHUNK[s]))
                while cur[s] < tgt and eligible(s, cur[s], xo_rows):
                    conv_chunk(cur[s], srcs, wts, R_out, R_in, evac, stag)
                    cur[s] += 1
                    emitted += 1
                    if emitted >= max_chunks:
                        return emitted
            return emitted

        # ================= attention branch =================
        dwt_n = max(dw_act_rows, BS - dw_pe_rows - dw_act_rows) * 264

        def emit_dw(br):
            slot = br % 2
            lo = 16 * br - 3
            rlo = max(0, -lo)
            nc.sync.dma_start(xchunk[:, slot, rlo:22, 4:260],
                              xbf[:, lo + rlo:lo + 22, :])
            # shifted copy: xodd[f] = xchunk[slot][f+1] (flat), so even-dx taps
            # get 4B-aligned starts for the DVE 2x/4x perf modes
            xcf = xchunk[:, slot].rearrange("c r j -> c (r j)")
            xof = xodd.rearrange("c r j -> c (r j)")
            NE = 23 * 264
            nc.vector.tensor_copy(xof[:, 0:NE - 1], xcf[:, 1:NE])
            # xa_r rows are 264 wide: data cols [4,260); the width pads absorb
            # the +-3 column shifts so every mul/add is one flat aligned run.
            xa_r = spool.tile([C, BS, 264], BF, tag="xar")
            xarf = xa_r.rearrange("c r j -> c (r j)")

            def flat_src(rfrom, rto, t):
                dyp, dxp = t // 7, t % 7
                n = (rto - rfrom) * 264
                s = (rfrom + dyp) * 264 + dxp - 3
                if dxp % 2 == 1:
                    return xcf[:, s:s + n]
                return xof[:, s - 1:s - 1 + n]

            # --- TensorE rows [0, dw_pe_rows): diagonal matmuls, per row
            for c0 in range(0, dw_pe_rows, 2):
                ps = cpsum.tile([C, 2, W], F32, tag="cps")
                nrr = min(2, dw_pe_rows - c0)
                mms = []
                for t in range(49):
                    dyp, dxp = t // 7, t % 7
                    for r in range(nrr):
                        mms.append((ps[:, r, :], diag[:, t, :],
                                    xchunk[:, slot, dyp + c0 + r,
                                           1 + dxp:1 + dxp + W], t == 0))
                for i, (o, wv, rhs, st) in enumerate(mms):
                    nc.tensor.matmul(o, wv, rhs, start=st,
                                     stop=(i == len(mms) - 1),
                                     skip_group_check=True)
                nc.scalar.activation(xa_r[:, c0:c0 + nrr, 4:260], ps[:, 0:nrr, :],
                                     AF.Identity, bias=bias_ap(2))
            # --- ScalarE mul + VectorE 4x add, rows [dw_pe_rows, a1)
            a0, a1 = dw_pe_rows, dw_pe_rows + dw_act_rows
            if dw_act_rows > 0:
                o = xarf[:, a0 * 264:a1 * 264]
                na = (a1 - a0) * 264
                for t in range(49):
                    if t == 0:
                        nc.scalar.activation(o, flat_src(a0, a1, t), AF.Copy,
                                             scale=awdw_sb[:, t:t + 1])
                        continue
                    tmp0 = spool.tile([C, dwt_n], BF, tag="dwt", name="dwt")
                    tmp = tmp0[:, :na]
                    nc.scalar.activation(tmp[:], flat_src(a0, a1, t), AF.Copy,
                                         scale=awdw_sb[:, t:t + 1])
                    nc.vector.tensor_tensor(o, o, tmp[:], ALU.add)
            # --- VectorE rows [a1, 16): 2x scaled-mul into tmp + 4x add
            # (the fused scalar_tensor_tensor runs at 1x; two ops are faster)
            if a1 < BS:
                o = xarf[:, a1 * 264:BS * 264]
                nv = (BS - a1) * 264
                for t in range(49):
                    if t == 0:
                        nc.vector.tensor_scalar_mul(o, flat_src(a1, BS, t),
                                                    awdw_sb[:, t:t + 1])
                        continue
                    vtmp0 = spool.tile([C, dwt_n], BF, tag="dwt", name="dwt")
                    vtmp = vtmp0[:, :nv]
                    nc.vector.tensor_scalar_mul(vtmp[:], flat_src(a1, BS, t),
                                                awdw_sb[:, t:t + 1])
                    nc.vector.tensor_tensor(o, o, vtmp[:], ALU.add)
            if a0 < BS:
                ob = xarf[:, a0 * 264:BS * 264]
                nc.vector.tensor_scalar_add(ob, ob, bias_ap(2))
            # relayout row-major -> block-major (pix = r*16+w within block)
            xa_t = apool.tile([C, BS, W], BF, tag="xa")
            nc.scalar.activation(
                xa_t.rearrange("c b (r w) -> c r b w", w=BS),
                xa_r[:, :, 4:260].rearrange("c r (b w) -> c r b w", w=BS), AF.Copy)
            return xa_t

        def emit_qk(br, xa_t):
            q_t = apool.tile([C, BS, W], BF, tag="q")  # block-major
            k_t = apool.tile([C, BS, W], BF, tag="k")
            for b0 in range(0, BS, 2):   # two blocks per matmul (N=512)
                rhs = xa_t[:, b0:b0 + 2, :]
                ps = cpsum.tile([C, 2, W], F32, tag="cps")
                nc.tensor.matmul(ps[:], aw["aw2"][:], rhs, start=True, stop=True)
                nc.scalar.activation(q_t[:, b0:b0 + 2, :], ps[:], AF.Identity,
                                     bias=bias_ap(3))
                ps2 = cpsum.tile([C, 2, W], F32, tag="cps")
                nc.tensor.matmul(ps2[:], aw["aw3"][:], rhs, start=True, stop=True)
                nc.scalar.activation(k_t[:, b0:b0 + 2, :], ps2[:], AF.Identity,
                                     bias=bias_ap(4))
            return q_t, k_t

        def attn_front(bc, xa_t, q_t, k_t):
            """logits + exp + xa-transposes for one block; returns handles."""
            lp = lpsum.tile([C, 2, W], F32, tag="lps")
            for h in (0, 1):
                nc.tensor.matmul(lp[:, h, :], k_t[:, bc, 128 * h:128 * h + 128],
                                 q_t[:, bc, :], start=True, stop=True,
                                 skip_group_check=(h == 1))
            atte = spool.tile([C, 2, W], BF, tag="atte")
            sums = spool.tile([C, 2], F32, tag="sums")
            for h in (0, 1):
                nc.scalar.activation(atte[:, h, :], lp[:, h, :], AF.Exp,
                                     accum_out=sums[:, h:h + 1])
            rsum = spool.tile([C, 2], F32, tag="rsum")
            nc.vector.reciprocal(rsum[:], sums[:])
            tp = tpsum.tile([C, 2, C], BF, tag="tps")
            for h in (0, 1):
                nc.tensor.transpose(tp[:, h, :],
                                    xa_t[:, bc, 128 * h:128 * h + 128],
                                    ident[:])
            xaTs = spool.tile([C, 2, C], BF, tag="xaTs")
            for h in (0, 1):
                nc.scalar.activation(xaTs[:, h, :], tp[:, h, :], AF.Copy,
                                     scale=rsum[:, h:h + 1])
            return atte, xaTs

        def attn_back(bc, atte, xaTs, xo_t):
            xp = xpsum.tile([C, W], F32, tag="xq")
            nc.tensor.matmul(xp[:], xaTs[:, 0, :], atte[:, 0, :],
                             start=True, stop=False)
            nc.tensor.matmul(xp[:], xaTs[:, 1, :], atte[:, 1, :],
                             start=False, stop=True)
            xats = spool.tile([C, W], BF, tag="xats")
            nc.scalar.activation(xats[:], xp[:], AF.Copy)
            op = xpsum.tile([C, W], F32, tag="xq")
            nc.tensor.matmul(op[:], aw["aw4"][:], xats[:], start=True, stop=True)
            # scatter back to row-major [16 rows, 16 cols] of this block
            nc.scalar.activation(xo_t[:, :, 16 * bc:16 * bc + 16], op[:],
                                 AF.Identity, bias=bias_ap(5))

        # ================= emission schedule =================
        # Attention for block-row br-1 runs while dw/qk of br grind on
        # VectorE/ScalarE; conv chunks interleave everywhere so TensorE
        # always has independent work during softmax/evac waits.
        def emit_attn(brA, xa_t, q_t, k_t, xo_rows, targets):
            xo_t = apool.tile([C, BS, W], BF, tag="xo")  # row-major
            pending = None
            for bc in range(16):
                fr = attn_front(bc, xa_t, q_t, k_t)
                if pending is not None:
                    attn_back(bc - 1, *pending, xo_t)
                pending = fr
                emit_convs(xo_rows, targets, max_chunks=3)
            attn_back(15, *pending, xo_t)
            nc.sync.dma_start(xo_d[:, 16 * brA:16 * brA + 16, :], xo_t[:])

        prev = None
        for br in range(NBR):
            frac = (br + 1) / NBR
            targets = {s: int(NCHUNK[s] * frac + 0.999) for s in NCHUNK}
            boost = 52 if br <= 1 else 12
            targets["c1"] = min(NCHUNK["c1"], targets["c1"] + boost)
            targets["c2"] = min(NCHUNK["c2"], targets["c2"] + boost)
            # attention of br-1 first: its ScalarE ops (softmax exp, evacs)
            # must precede this block-row's dw mul chain in the ACT queue
            if prev is not None:
                emit_attn(prev[0], prev[1], prev[2], prev[3], 16 * br - 16, targets)
            xa_t = emit_dw(br)
            emit_convs(16 * br - 16, targets, max_chunks=52 if br == 0 else 8)
            q_t, k_t = emit_qk(br, xa_t)
            prev = (br, xa_t, q_t, k_t)
            emit_convs(16 * br - 16, targets)
        # xo rows of the final block-row are not in DRAM until emit_attn's
        # closing DMA is emitted, so interleaved chunks may only rely on
        # rows < 16*prev[0] here; the drain below picks up the rest.
        emit_attn(prev[0], prev[1], prev[2], prev[3], 16 * prev[0], {})
        # drain remaining conv chunks (dependencies resolve progressively)
        for _ in range(12):
            before = dict(cur)
            emit_convs(NBR * BS, {})
            if cur == before:
                break
        assert all(cur[s] == NCHUNK[s] for s in NCHUNK), cur


# ------------------------- host-side shard prep -------------------------
B, H = 4, 256

bf16 = ml_dtypes.bfloat16


def _prep_weights(flip):
    """Returns dict of weight arrays for one orientation (flip=True for
    bottom-half cores: conv kernels y-flipped)."""
    return flip


def make_in_maps(d):
    """d: dict of full inputs (numpy f32). Returns list of 8 per-core maps."""
    x = np.asarray(d['x'])
    scale = C ** (-0.5)

    def conv_taps(w, flip):
        # w [O, I, 3, 3] -> [9, I, O] lhsT per tap
        ww = w[:, :, ::-1, :] if flip else w
        return np.ascontiguousarray(
            ww.transpose(2, 3, 1, 0).reshape(9, w.shape[1], w.shape[0])
        ).astype(bf16)

    per_orient = {}
    for flip in (False, True):
        fw = np.asarray(d['fw'])
        aw1 = np.asarray(d['aw1'])[:, 0]  # [C,7,7]
        if flip:
            aw1 = aw1[:, ::-1, :]
        per_orient[flip] = {
            'w_r1w1': conv_taps(np.asarray(d['r1w1']), flip),
            'w_r1w2': conv_taps(np.asarray(d['r1w2']), flip),
            'w_fw1': conv_taps(fw[:, :C], flip),
            'w_fw2': conv_taps(fw[:, C:], flip),
            'w_r2w1': conv_taps(np.asarray(d['r2w1']), flip),
            'w_r2w2': conv_taps(np.asarray(d['r2w2']), flip),
            'awdw': np.ascontiguousarray(aw1.reshape(C, 49)).astype(np.float32),
        }
    shared = {
        'w_aw2': np.ascontiguousarray(np.asarray(d['aw2']).T).astype(bf16),
        'w_aw3': np.ascontiguousarray(np.asarray(d['aw3']).T * scale).astype(bf16),
        'w_aw4': np.ascontiguousarray(np.asarray(d['aw4']).T).astype(bf16),
        'biases': np.stack([np.asarray(d[k]) if k != 'ab3s'
                            else np.asarray(d['ab3']) * scale
                            for k in ('r1b1', 'r1b2', 'ab1', 'ab2', 'ab3s',
                                      'ab4', 'fb', 'r2b1', 'r2b2')],
                           axis=1).astype(np.float32),
    }
    in_maps = []
    for core in range(8):
        b, j = core // 2, core % 2
        if j == 0:
            xs = x[b, :, :R_X, :]
            xr = x[b, :, :R_OUT, :]
        else:
            xf = x[b, :, ::-1, :]
            xs = xf[:, :R_X, :]
            xr = xf[:, :R_OUT, :]
        m = {
            'xbf': np.ascontiguousarray(xs).astype(bf16),
            'xres': np.ascontiguousarray(xr).astype(np.float32),
        }
        m.update(per_orient[j == 1])
        m.update(shared)
        in_maps.append(m)
    return in_maps


def assemble_output(results):
    """results: list of 8 dicts with 'out' [C, 128, W] f32."""
    out = np.empty((B, C, H, W), np.float32)
    for core in range(8):
        b, j = core // 2, core % 2
        o = results[core]['out']
        if j == 0:
            out[b, :, :R_OUT, :] = o
        else:
            out[b, :, H - R_OUT:, :] = o[:, ::-1, :]
    return out


# ------------------------- entry point -------------------------
_CACHED = {}

def kernel(**inputs):
    if "nc" not in _CACHED:
        _CACHED["nc"] = build()
    nc = _CACHED["nc"]
    in_maps = make_in_maps(inputs)
    res = run_bass_kernel_spmd(nc, in_maps, core_ids=list(range(8)))
    return assemble_output(res.results)

